# revision 12
# baseline (speedup 1.0000x reference)
"""Dense language-guidance cross-attention kernel for 8 Trainium2 cores.

Math (per batch b):
    K_l = lang @ W_lk.T + b_lk           (N, C)
    V_l = lang @ W_lv.T + b_lv           (N, C)
    A   = softmax_n(vis @ W_vk.T @ K_l.T / sqrt(C))   (S, N)
    out = A @ V_l + A @ (A.T @ (vis @ W_vv.T + b_vv)) (S, C)

Sharding: data-parallel over B — core i computes batch i end-to-end.

Key restructure vs the naive dataflow: the two S x C x C projections
(K_v = vis @ W_vk.T, V_v = vis @ W_vv.T, 8.6 GFLOP each) are eliminated
by reassociating through the tiny N=77 language dim:

  logits = K_v @ K_l.T = vis @ M.T + c      M = K_l @ (s*W_vk)  (N, C)
                                            c[n] = K_l[n] . (s*b_vk)
  X = A.T @ V_v = Y @ W_vv.T + g (x) b_vv   Y = A.T @ vis       (N, C)
                                            g[n] = sum_s A[s, n]
  out = (E @ (V_l + X)) / Z                 E = exp(logits), Z = row-sum

This drops per-core FLOPs from 19.4G to 2.4G; the kernel becomes
DMA/overlap-limited instead of PE-limited.

Device details:
  * 1/sqrt(C) folded into W_vk/b_vk on host (exact: C**-0.5 == 2**-5).
  * softmax without max-subtraction (logits ~N(0, 0.34)); E = exp(logits)
    unnormalized, Z via transpose-block reduce, A = E/Z per 128-row block.
  * c[n] enters as the per-partition bias of the Exp activation.
  * all transposes use a [77, x] input with eye[:77, :77] so no padding
    rows/cols ever need zeroing; DVE reduces/scales straight from PSUM.
  * g accumulated in PSUM via ones-column matmuls; its b_vv contribution
    enters the X PSUM as a rank-1 matmul (g^T stationary, b_vv row moving).
  * vis is DMA'd twice in the two layouts the two contractions need:
    visT [c, s] (moving for logits) and visN [s, c] (moving for Y).
  * V_l is computed after pass 1 so the prologue never waits on W_lv.
  * everything on PE runs fp16 (full-rate + fast weight load); output is
    written fp16 and upcast on host (absmax-rel error ~5e-4 total).
"""

import numpy as np

B, S, N, C = 8, 4096, 77, 1024
P = 128
CT = C // P          # 8 tiles over the feature dim
SCHUNK = 512         # s-chunk processed per main-loop iteration
NCHUNKS = S // SCHUNK
SBLK = SCHUNK // P   # 128-row blocks per chunk
NCORES = 8

_prog_cache = {}


def _build_program():
    if "nc" in _prog_cache:
        return _prog_cache["nc"]

    import concourse.bacc as bacc
    import concourse.mybir as mybir
    import concourse.tile as tile

    fp32 = mybir.dt.float32
    f16 = mybir.dt.float16  # fp16: full-rate PE + FWL
    bf16 = mybir.dt.bfloat16
    EXP = mybir.ActivationFunctionType.Exp
    COPY = mybir.ActivationFunctionType.Copy

    nc = bacc.Bacc()

    visT = nc.declare_dram_parameter("visT", [C, S], f16, isOutput=False)
    visN = nc.declare_dram_parameter("visN", [S, C], f16, isOutput=False)
    langT = nc.declare_dram_parameter("langT", [C, N], f16, isOutput=False)
    wlkT = nc.declare_dram_parameter("wlkT", [C, C], f16, isOutput=False)
    wvkN = nc.declare_dram_parameter("wvkN", [C, C], f16, isOutput=False)
    wvvT = nc.declare_dram_parameter("wvvT", [C, C], f16, isOutput=False)
    wlvT = nc.declare_dram_parameter("wlvT", [C, C], f16, isOutput=False)
    bvk_c = nc.declare_dram_parameter("bvk_c", [P, CT], f16, isOutput=False)
    blk_c = nc.declare_dram_parameter("blk_c", [P, CT], fp32, isOutput=False)
    blv_b = nc.declare_dram_parameter("blv_b", [P, C], fp32, isOutput=False)
    bvv_r = nc.declare_dram_parameter("bvv_r", [1, C], f16, isOutput=False)
    ones_h = nc.declare_dram_parameter("ones_h", [P, 1], f16, isOutput=False)
    eye_d = nc.declare_dram_parameter("eye", [P, P], f16, isOutput=False)
    out_d = nc.declare_dram_parameter("out", [S, C], f16, isOutput=True)

    # [r, x] -> [p, t, x] with r = t*128 + p
    visT_r = visT.rearrange("(t p) s -> p t s", p=P)
    visN_r = visN.rearrange("(t p) c -> p t c", p=P)
    langT_r = langT.rearrange("(t p) n -> p t n", p=P)
    wlkT_r = wlkT.rearrange("(t p) d -> p t d", p=P)
    wvkN_r = wvkN.rearrange("(t p) c -> p t c", p=P)
    wvvT_r = wvvT.rearrange("(t p) c -> p t c", p=P)
    wlvT_r = wlvT.rearrange("(t p) c -> p t c", p=P)

    with tile.TileContext(nc) as tc, \
         tc.tile_pool(name="wbig", bufs=1) as wbig, \
         tc.tile_pool(name="io", bufs=3) as io, \
         tc.tile_pool(name="io2", bufs=3) as io2, \
         tc.tile_pool(name="persist", bufs=1) as persist, \
         tc.tile_pool(name="expat", bufs=NCHUNKS) as expat_pool, \
         tc.tile_pool(name="work", bufs=3) as work, \
         tc.tile_pool(name="psY", bufs=2, space="PSUM") as psY, \
         tc.tile_pool(name="psO", bufs=3, space="PSUM") as psO, \
         tc.tile_pool(name="psT", bufs=2, space="PSUM") as psT, \
         tc.tile_pool(name="psG", bufs=1, space="PSUM") as psG:

        def absorb(ap):
            """Standalone LDWEIGHTS that takes over a freshly-DMA'd tile's
            sem wait on the PE (fp16 matmuls lower to LDWEIGHTS+MATMUL whose
            LW slot carries at most ONE sync wait)."""
            nc.tensor.ldweights(ap.bitcast(bf16)[:, :64])

        # ---- constants / small inputs --------------------------------
        eye = persist.tile([P, P], f16)
        nc.sync.dma_start(out=eye[:], in_=eye_d[:])
        bvk = persist.tile([P, CT], f16)
        nc.sync.dma_start(out=bvk[:], in_=bvk_c[:])
        blk = persist.tile([P, CT], fp32)
        nc.sync.dma_start(out=blk[:], in_=blk_c[:])
        blv = persist.tile([P, C], fp32)
        nc.sync.dma_start(out=blv[:], in_=blv_b[:])
        bvv = persist.tile([1, C], f16)
        nc.sync.dma_start(out=bvv[:], in_=bvv_r[:])
        ones = persist.tile([P, 1], f16)
        nc.sync.dma_start(out=ones[:], in_=ones_h[:])
        lT = persist.tile([P, CT, P], f16)
        nc.vector.memset(lT[:].bitcast(fp32), 0.0)
        nc.sync.dma_start(out=lT[:, :, :N], in_=langT_r[:])

        absorb(lT[:, 0, :])
        absorb(eye[:, :])
        # DVE touches: absorb the bias tiles' DMA-queue waits onto the DVE
        # proc so bias-fused copyouts never carry a second (external) wait.
        dve_touch = persist.tile([P, 2], fp32)
        nc.vector.tensor_copy(dve_touch[:, 0:1], blk[:, 0:1])
        nc.vector.tensor_copy(dve_touch[:, 1:2], blv[:, 0:1])

        # ---- prologue weights (split across queues) ------------------
        wlk = wbig.tile([P, CT, C], f16)
        for k in range(CT):
            nc.sync.dma_start(out=wlk[:, k, :], in_=wlkT_r[:, k, :])
        wvk = wbig.tile([P, CT, C], f16)
        for k in range(CT):
            nc.scalar.dma_start(out=wvk[:, k, :], in_=wvkN_r[:, k, :])
        absorb(wlk[:, 0, :])
        absorb(wvk[:, 0, :])

        # prefetch first vis chunks ahead of the epilogue weight slabs
        vt_pre, vn_pre = [], []
        for ch in range(3):
            nsp = 4 if ch == 0 else 2
            vtp = io.tile([P, CT, SCHUNK], f16, name="vis_chunk", tag="vis_chunk")
            for k in range(0, CT, CT // nsp):
                kk = CT // nsp
                nc.sync.dma_start(
                    out=vtp[:, k:k + kk, :],
                    in_=visT_r[:, k:k + kk, ch * SCHUNK:(ch + 1) * SCHUNK])
            vnp = io2.tile([P, SBLK, C], f16, name="visn_chunk", tag="visn_chunk")
            for b in range(0, SBLK, SBLK // nsp if nsp <= SBLK else 1):
                bb = max(SBLK // nsp, 1)
                nc.sync.dma_start(out=vnp[:, b:b + bb, :],
                                  in_=visN_r[:, ch * SBLK + b:ch * SBLK + b + bb, :])
            absorb(vtp[:, 0, :])
            absorb(vnp[:, 0, :])
            vt_pre.append(vtp)
            vn_pre.append(vnp)

        # epilogue weights (needed only after pass 1)
        wlv = wbig.tile([P, CT, C], f16)
        for k in range(0, CT, 2):
            nc.scalar.dma_start(out=wlv[:, k:k + 2, :], in_=wlvT_r[:, k:k + 2, :])
        wvv = wbig.tile([P, CT, C], f16)
        for k in range(0, CT, 2):
            nc.sync.dma_start(out=wvv[:, k:k + 2, :], in_=wvvT_r[:, k:k + 2, :])
        absorb(wlv[:, 0, :])
        absorb(wvv[:, 0, :])

        # ---- prologue: K_l^T, c, M^T ---------------------------------
        # K_l natural [n, d] via wide matmuls, then PE-transpose to
        # klT [d, n] tiles (+b_lk per-partition on the transposed copyout).
        kl_sb = persist.tile([P, C], f16)
        for cc in range(2):
            sl = slice(cc * 512, (cc + 1) * 512)
            kps = psO.tile([P, 512], fp32, name="ps_kl", tag="acc512")
            for ct in range(CT):
                nc.tensor.matmul(
                    kps[:N, :], lT[:, ct, :N], wlk[:, ct, sl],
                    start=(ct == 0), stop=(ct == CT - 1),
                )
            nc.vector.tensor_copy(kl_sb[:N, sl], kps[:N, :])
        klT = persist.tile([P, CT, N], f16)
        for dt in range(CT):
            pst = psT.tile([P, N], f16, name="pst_a", tag="tp")
            nc.tensor.transpose(
                pst[:, :], kl_sb[:N, dt * P:(dt + 1) * P], eye[:N, :N])
            nc.vector.tensor_tensor(
                klT[:, dt, :], pst[:, :],
                blk[:, dt:dt + 1].to_broadcast([P, N]), mybir.AluOpType.add)

        # c[n] = K_l[n] . b_vk  -> cT [n, 1] fp32
        cps = psT.tile([P, 1], fp32, name="ps_c", tag="tp")
        for dt in range(CT):
            nc.tensor.matmul(
                cps[:N, :], klT[:, dt, :N], bvk[:, dt:dt + 1],
                start=(dt == 0), stop=(dt == CT - 1),
            )
        cT = persist.tile([P, 1], fp32)
        nc.vector.tensor_copy(cT[:N, :], cps[:N, :])

        # M natural [n, c] = K_l @ (s*W_vk) via wide matmuls, then
        # PE-transpose to mT [c, n] tiles.
        m_sb = persist.tile([P, C], f16)
        for cc in range(2):
            sl = slice(cc * 512, (cc + 1) * 512)
            mps = psO.tile([P, 512], fp32, name="ps_m", tag="acc512")
            for dt in range(CT):
                nc.tensor.matmul(
                    mps[:N, :], klT[:, dt, :N], wvk[:, dt, sl],
                    start=(dt == 0), stop=(dt == CT - 1),
                )
            nc.vector.tensor_copy(m_sb[:N, sl], mps[:N, :])
        mT = persist.tile([P, CT, N], f16)
        for ct in range(CT):
            pst = psT.tile([P, N], f16, name="pst_m", tag="tp")
            nc.tensor.transpose(
                pst[:, :], m_sb[:N, ct * P:(ct + 1) * P], eye[:N, :N])
            nc.vector.tensor_copy(mT[:, ct, :], pst[:, :])

        # ---- persistent accumulators ---------------------------------
        rz_all = persist.tile([P, S // P], fp32)   # 1/Z, [s mod 128, s // 128]
        yp0 = psY.tile([P, 512], fp32, name="ps_y0", tag="ydd")
        yp1 = psY.tile([P, 512], fp32, name="ps_y1", tag="ydd")
        gps = psG.tile([P, 1], fp32, name="ps_g", tag="g")

        expat_tiles = []

        # ================= pass 1: over s-chunks ======================
        for ch in range(NCHUNKS):
            s0 = ch * SCHUNK
            if ch < 3:
                vt, vn = vt_pre[ch], vn_pre[ch]
            else:
                vt = io.tile([P, CT, SCHUNK], f16, name="vis_chunk",
                             tag="vis_chunk")
                for k in range(0, CT, 4):
                    nc.sync.dma_start(out=vt[:, k:k + 4, :],
                                      in_=visT_r[:, k:k + 4, s0:s0 + SCHUNK])
                vn = io2.tile([P, SBLK, C], f16, name="visn_chunk",
                              tag="visn_chunk")
                for b in range(0, SBLK, 2):
                    nc.sync.dma_start(
                        out=vn[:, b:b + 2, :],
                        in_=visN_r[:, ch * SBLK + b:ch * SBLK + b + 2, :])
                absorb(vt[:, 0, :])
                absorb(vn[:, 0, :])

            # logits [n, s-chunk]
            lg = psO.tile([P, SCHUNK], fp32, name="ps_logits", tag="acc512")
            for k in range(CT):
                nc.tensor.matmul(
                    lg[:N, :], mT[:, k, :N], vt[:, k, :],
                    start=(k == 0), stop=(k == CT - 1),
                    skip_group_check=True,
                )

            # E = exp(logits + c) kept resident for pass 2
            ea = expat_pool.tile([P, SCHUNK], f16, name="expat")
            nc.scalar.activation(ea[:N, :], lg[:N, :], EXP, bias=cT[:N, 0:1])

            # per 128-row block: transpose -> [s, n], Z, 1/Z, A=E/Z,
            # then Y += A-block^T-stationary @ visN, g += A @ 1
            for b in range(SBLK):
                pst = psT.tile([P, N], f16, name="pst_a", tag="tp")
                nc.tensor.transpose(
                    pst[:, :], ea[:N, b * P:(b + 1) * P], eye[:N, :N]
                )
                zcol = work.tile([P, 1], fp32, name="zcol")
                nc.vector.reduce_sum(zcol[:], pst[:, :], axis=mybir.AxisListType.X)
                rzc = rz_all[:, ch * SBLK + b: ch * SBLK + b + 1]
                nc.vector.reciprocal(rzc, zcol[:])
                an = work.tile([P, N], f16, name="a_norm")
                nc.vector.tensor_tensor(
                    an[:, :], pst[:, :], rzc.to_broadcast([P, N]),
                    mybir.AluOpType.mult)
                first = (ch == 0 and b == 0)
                last = (ch == NCHUNKS - 1 and b == SBLK - 1)
                nc.tensor.matmul(
                    yp0[:N, :], an[:, :], vn[:, b, 0:512],
                    start=first, stop=last, skip_group_check=True)
                nc.tensor.matmul(
                    yp1[:N, :], an[:, :], vn[:, b, 512:1024],
                    start=first, stop=last, skip_group_check=True)
                nc.tensor.matmul(
                    gps[:N, :], an[:, :], ones[:, :],
                    start=first, stop=last, skip_group_check=True)

            expat_tiles.append(ea)

            if ch == 5:
                # V_l [n, c] natural, +b_lv — interleaved here so the
                # epilogue never waits on it (W_lv arrived mid-pass-1)
                vl = persist.tile([P, C], fp32)
                for cc in range(2):
                    sl = slice(cc * 512, (cc + 1) * 512)
                    vps = psO.tile([P, 512], fp32, name="ps_vl", tag="acc512")
                    for dt in range(CT):
                        nc.tensor.matmul(
                            vps[:N, :], lT[:, dt, :N], wlv[:, dt, sl],
                            start=(dt == 0), stop=(dt == CT - 1),
                            skip_group_check=True,
                        )
                    nc.vector.tensor_add(vl[:N, sl], vps[:N, :], blv[:N, sl])

        # ============ epilogue: X = Y @ W_vv.T + g (x) b_vv ===========
        # Y psum -> SBUF, transpose to [d, n]
        y_sb = persist.tile([P, C], f16)
        nc.vector.tensor_copy(y_sb[:N, 0:512], yp0[:N, :])
        nc.vector.tensor_copy(y_sb[:N, 512:1024], yp1[:N, :])
        g_sb = persist.tile([P, 1], f16)
        nc.vector.tensor_copy(g_sb[:N, :], gps[:N, :])

        yT = persist.tile([P, CT, N], f16)
        for t in range(CT):
            pst = psT.tile([P, N], f16, name="pst_a", tag="tp")
            nc.tensor.transpose(
                pst[:, :], y_sb[:N, t * P:(t + 1) * P], eye[:N, :N]
            )
            nc.vector.tensor_copy(yT[:, t, :], pst[:, :])
        pstg = psT.tile([P, N], f16, name="pst_a", tag="tp")
        nc.tensor.transpose(pstg[:1, :], g_sb[:N, 0:1], eye[:N, :N])
        gT = persist.tile([1, N], f16)
        nc.vector.tensor_copy(gT[:1, :], pstg[:1, :])

        wx = persist.tile([P, C], f16)
        for cc in range(2):
            sl = slice(cc * 512, (cc + 1) * 512)
            xps = psY.tile([P, 512], fp32, name="ps_x", tag="ydd")
            for t in range(CT):
                nc.tensor.matmul(
                    xps[:N, :], yT[:, t, :N], wvv[:, t, sl],
                    start=(t == 0), stop=False, skip_group_check=True,
                )
            nc.tensor.matmul(
                xps[:N, :], gT[:1, :N], bvv[:1, sl],
                start=False, stop=True, skip_group_check=True,
            )
            # wx = V_l + X
            nc.vector.tensor_tensor(
                wx[:N, sl], vl[:N, sl], xps[:N, :], mybir.AluOpType.add)

        # ================= pass 2: out = (E @ wx) / Z =================
        for ch in range(NCHUNKS):
            ea = expat_tiles[ch]
            for b in range(SBLK):
                rzc = rz_all[:, ch * SBLK + b: ch * SBLK + b + 1]
                r0 = ch * SCHUNK + b * P
                mid = work.tile([P, C], f16, name="mid_out", bufs=4)
                for cc in range(2):
                    sl = slice(cc * 512, (cc + 1) * 512)
                    ops_ = psO.tile([P, 512], fp32, name="ps_out", tag="acc512")
                    nc.tensor.matmul(
                        ops_[:, :], ea[:N, b * P:(b + 1) * P], wx[:N, sl],
                        start=True, stop=True,
                    )
                    # 1/Z scale on the copyout; alternate ACT/DVE
                    if cc == 0:
                        nc.scalar.activation(mid[:, sl], ops_[:, :], COPY,
                                             scale=rzc)
                    else:
                        nc.vector.tensor_tensor(
                            mid[:, sl], ops_[:, :], rzc.to_broadcast([P, 512]),
                            mybir.AluOpType.mult)
                    # 128KB store per half; alternate queue sets for
                    # transfer parallelism
                    eng = nc.sync if cc == 0 else nc.gpsimd
                    eng.dma_start(out=out_d[r0:r0 + P, sl], in_=mid[:, sl])

    nc.compile()
    _prog_cache["nc"] = nc
    return nc


def _make_in_maps(inputs):
    vis_features = inputs["vis_features"]
    lang_features = inputs["lang_features"]
    W_vk, b_vk = inputs["W_vk"], inputs["b_vk"]
    W_lk, b_lk = inputs["W_lk"], inputs["b_lk"]
    W_vv, b_vv = inputs["W_vv"], inputs["b_vv"]
    W_lv, b_lv = inputs["W_lv"], inputs["b_lv"]
    assert vis_features.shape == (B, S, C) and lang_features.shape == (B, N, C)

    f = np.float32
    scale = f(C) ** f(-0.5)  # 2**-5, exact
    h = np.float16
    wlkTh = np.ascontiguousarray(W_lk.T.astype(f)).astype(h)
    wvkNh = np.ascontiguousarray((W_vk * scale).astype(f)).astype(h)
    wvvTh = np.ascontiguousarray(W_vv.T.astype(f)).astype(h)
    wlvTh = np.ascontiguousarray(W_lv.T.astype(f)).astype(h)
    bvk_c = np.ascontiguousarray((b_vk * scale).astype(f).reshape(CT, P).T).astype(h)
    blk_c = np.ascontiguousarray(b_lk.astype(f).reshape(CT, P).T)
    blv_b = np.ascontiguousarray(np.broadcast_to(b_lv.astype(f), (P, C)))
    bvv_r = b_vv.astype(f).reshape(1, C).astype(h)
    ones_col = np.ones((P, 1), dtype=h)
    eye = np.eye(P, dtype=h)

    shared = dict(wlkT=wlkTh, wvkN=wvkNh, wvvT=wvvTh, wlvT=wlvTh,
                  bvk_c=bvk_c, blk_c=blk_c, blv_b=blv_b, bvv_r=bvv_r,
                  ones_h=ones_col, eye=eye)
    in_maps = []
    for b in range(B):
        m = dict(shared)
        vb = vis_features[b].astype(f)
        m["visT"] = np.ascontiguousarray(vb.T).astype(h)
        m["visN"] = np.ascontiguousarray(vb).astype(h)
        m["langT"] = np.ascontiguousarray(lang_features[b].T.astype(f)).astype(h)
        in_maps.append(m)
    return in_maps


def kernel(**inputs):
    in_maps = _make_in_maps(inputs)
    nc = _build_program()
    from concourse.bass_utils import run_bass_kernel_spmd
    res = run_bass_kernel_spmd(nc, in_maps, list(range(NCORES)))
    return np.stack(
        [res.results[i]["out"].astype(np.float32) for i in range(NCORES)],
        axis=0)


# revision 13
# speedup vs baseline: 1.0773x; 1.0773x over previous
"""Dense language-guidance cross-attention kernel for 8 Trainium2 cores.

Math (per batch b):
    K_l = lang @ W_lk.T + b_lk           (N, C)
    V_l = lang @ W_lv.T + b_lv           (N, C)
    A   = softmax_n(vis @ W_vk.T @ K_l.T / sqrt(C))   (S, N)
    out = A @ V_l + A @ (A.T @ (vis @ W_vv.T + b_vv)) (S, C)

Sharding: data-parallel over B — core i computes batch i end-to-end.

Key restructure vs the naive dataflow: the two S x C x C projections
(K_v = vis @ W_vk.T, V_v = vis @ W_vv.T, 8.6 GFLOP each) are eliminated
by reassociating through the tiny N=77 language dim:

  logits = K_v @ K_l.T = vis @ M.T + c      M = K_l @ (s*W_vk)  (N, C)
                                            c[n] = K_l[n] . (s*b_vk)
  X = A.T @ V_v = Y @ W_vv.T + g (x) b_vv   Y = A.T @ vis       (N, C)
                                            g[n] = sum_s A[s, n]
  out = (E @ (V_l + X)) / Z                 E = exp(logits), Z = row-sum

This drops per-core FLOPs from 19.4G to 2.4G; the kernel becomes
DMA/overlap-limited instead of PE-limited.

Device details:
  * 1/sqrt(C) folded into W_vk/b_vk on host (exact: C**-0.5 == 2**-5).
  * softmax without max-subtraction (logits ~N(0, 0.34)); E = exp(logits)
    unnormalized, Z via transpose-block reduce, A = E/Z per 128-row block.
  * c[n] enters as the per-partition bias of the Exp activation.
  * all transposes use a [77, x] input with eye[:77, :77] so no padding
    rows/cols ever need zeroing; DVE reduces/scales straight from PSUM.
  * g accumulated in PSUM via ones-column matmuls; its b_vv contribution
    enters the X PSUM as a rank-1 matmul (g^T stationary, b_vv row moving).
  * vis is DMA'd twice in the two layouts the two contractions need:
    visT [c, s] (moving for logits) and visN [s, c] (moving for Y).
  * V_l is computed after pass 1 so the prologue never waits on W_lv.
  * everything on PE runs fp16 (full-rate + fast weight load); output is
    written fp16 and upcast on host (absmax-rel error ~5e-4 total).
"""

import numpy as np

B, S, N, C = 8, 4096, 77, 1024
P = 128
CT = C // P          # 8 tiles over the feature dim
SCHUNK = 512         # s-chunk processed per main-loop iteration
NCHUNKS = S // SCHUNK
SBLK = SCHUNK // P   # 128-row blocks per chunk
NCORES = 8

_prog_cache = {}


def _build_program():
    if "nc" in _prog_cache:
        return _prog_cache["nc"]

    import concourse.bacc as bacc
    import concourse.mybir as mybir
    import concourse.tile as tile

    fp32 = mybir.dt.float32
    f16 = mybir.dt.float16  # fp16: full-rate PE + FWL
    bf16 = mybir.dt.bfloat16
    EXP = mybir.ActivationFunctionType.Exp
    COPY = mybir.ActivationFunctionType.Copy

    nc = bacc.Bacc()

    visT = nc.declare_dram_parameter("visT", [C, S], f16, isOutput=False)
    visN = nc.declare_dram_parameter("visN", [S, C], f16, isOutput=False)
    langT = nc.declare_dram_parameter("langT", [C, N], f16, isOutput=False)
    wlkT = nc.declare_dram_parameter("wlkT", [C, C], f16, isOutput=False)
    wvkN = nc.declare_dram_parameter("wvkN", [C, C], f16, isOutput=False)
    wvvT = nc.declare_dram_parameter("wvvT", [C, C], f16, isOutput=False)
    wlvT = nc.declare_dram_parameter("wlvT", [C, C], f16, isOutput=False)
    bvk_c = nc.declare_dram_parameter("bvk_c", [P, CT], f16, isOutput=False)
    blk_c = nc.declare_dram_parameter("blk_c", [P, CT], fp32, isOutput=False)
    blv_b = nc.declare_dram_parameter("blv_b", [P, C], fp32, isOutput=False)
    bvv_r = nc.declare_dram_parameter("bvv_r", [1, C], f16, isOutput=False)
    ones_h = nc.declare_dram_parameter("ones_h", [P, 1], f16, isOutput=False)
    eye_d = nc.declare_dram_parameter("eye", [P, P], f16, isOutput=False)
    out_d = nc.declare_dram_parameter("out", [S, C], f16, isOutput=True)

    # [r, x] -> [p, t, x] with r = t*128 + p
    visT_r = visT.rearrange("(t p) s -> p t s", p=P)
    visN_r = visN.rearrange("(t p) c -> p t c", p=P)
    langT_r = langT.rearrange("(t p) n -> p t n", p=P)
    wlkT_r = wlkT.rearrange("(t p) d -> p t d", p=P)
    wvkN_r = wvkN.rearrange("(t p) c -> p t c", p=P)
    wvvT_r = wvvT.rearrange("(t p) c -> p t c", p=P)
    wlvT_r = wlvT.rearrange("(t p) c -> p t c", p=P)

    with tile.TileContext(nc) as tc, \
         tc.tile_pool(name="wbig", bufs=1) as wbig, \
         tc.tile_pool(name="io", bufs=3) as io, \
         tc.tile_pool(name="io2", bufs=3) as io2, \
         tc.tile_pool(name="persist", bufs=1) as persist, \
         tc.tile_pool(name="expat", bufs=NCHUNKS) as expat_pool, \
         tc.tile_pool(name="work", bufs=3) as work, \
         tc.tile_pool(name="psY", bufs=2, space="PSUM") as psY, \
         tc.tile_pool(name="psO", bufs=3, space="PSUM") as psO, \
         tc.tile_pool(name="psT", bufs=2, space="PSUM") as psT, \
         tc.tile_pool(name="psG", bufs=1, space="PSUM") as psG:

        def absorb(ap):
            """Standalone LDWEIGHTS that takes over a freshly-DMA'd tile's
            sem wait on the PE (fp16 matmuls lower to LDWEIGHTS+MATMUL whose
            LW slot carries at most ONE sync wait)."""
            nc.tensor.ldweights(ap.bitcast(bf16)[:, :64])

        # ---- critical-path inputs first: langT + W_lk + W_vk ---------
        lT = persist.tile([P, CT, P], f16)
        nc.vector.memset(lT[:].bitcast(fp32), 0.0)
        nc.sync.dma_start(out=lT[:, :, :N], in_=langT_r[:])
        wlk = wbig.tile([P, CT, C], f16)
        for k in range(CT):
            nc.sync.dma_start(out=wlk[:, k, :], in_=wlkT_r[:, k, :])
        wvk = wbig.tile([P, CT, C], f16)
        for k in range(CT):
            nc.scalar.dma_start(out=wvk[:, k, :], in_=wvkN_r[:, k, :])
        absorb(lT[:, 0, :])
        absorb(wlk[:, 0, :])
        absorb(wvk[:, 0, :])

        # ---- small constants (off the critical path) -----------------
        blk = persist.tile([P, CT], fp32)
        nc.sync.dma_start(out=blk[:], in_=blk_c[:])
        eye = persist.tile([P, P], f16)
        nc.sync.dma_start(out=eye[:], in_=eye_d[:])
        bvk = persist.tile([P, CT], f16)
        nc.sync.dma_start(out=bvk[:], in_=bvk_c[:])
        blv = persist.tile([P, C], fp32)
        nc.sync.dma_start(out=blv[:], in_=blv_b[:])
        bvv = persist.tile([1, C], f16)
        nc.sync.dma_start(out=bvv[:], in_=bvv_r[:])
        ones = persist.tile([P, 1], f16)
        nc.sync.dma_start(out=ones[:], in_=ones_h[:])

        absorb(eye[:, :])
        # DVE touches: absorb the bias tiles' DMA-queue waits onto the DVE
        # proc so bias-fused copyouts never carry a second (external) wait.
        dve_touch = persist.tile([P, 2], fp32)
        nc.vector.tensor_copy(dve_touch[:, 0:1], blk[:, 0:1])
        nc.vector.tensor_copy(dve_touch[:, 1:2], blv[:, 0:1])

        # prefetch first vis chunks ahead of the epilogue weight slabs
        vt_pre, vn_pre = [], []
        for ch in range(3):
            nsp = 4 if ch == 0 else 2
            vtp = io.tile([P, CT, SCHUNK], f16, name="vis_chunk", tag="vis_chunk")
            for k in range(0, CT, CT // nsp):
                kk = CT // nsp
                nc.sync.dma_start(
                    out=vtp[:, k:k + kk, :],
                    in_=visT_r[:, k:k + kk, ch * SCHUNK:(ch + 1) * SCHUNK])
            vnp = io2.tile([P, SBLK, C], f16, name="visn_chunk", tag="visn_chunk")
            for b in range(0, SBLK, SBLK // nsp if nsp <= SBLK else 1):
                bb = max(SBLK // nsp, 1)
                nc.scalar.dma_start(out=vnp[:, b:b + bb, :],
                                    in_=visN_r[:, ch * SBLK + b:ch * SBLK + b + bb, :])
            absorb(vtp[:, 0, :])
            absorb(vnp[:, 0, :])
            vt_pre.append(vtp)
            vn_pre.append(vnp)

        # epilogue weights (needed only after pass 1)
        wlv = wbig.tile([P, CT, C], f16)
        for k in range(0, CT, 2):
            nc.scalar.dma_start(out=wlv[:, k:k + 2, :], in_=wlvT_r[:, k:k + 2, :])
        wvv = wbig.tile([P, CT, C], f16)
        for k in range(0, CT, 2):
            nc.sync.dma_start(out=wvv[:, k:k + 2, :], in_=wvvT_r[:, k:k + 2, :])
        absorb(wlv[:, 0, :])
        absorb(wvv[:, 0, :])

        # ---- prologue: K_l^T, c, M^T ---------------------------------
        # K_l natural [n, d] via wide matmuls, then PE-transpose to
        # klT [d, n] tiles (+b_lk per-partition on the transposed copyout).
        kl_sb = persist.tile([P, C], f16)
        for cc in range(2):
            sl = slice(cc * 512, (cc + 1) * 512)
            kps = psO.tile([P, 512], fp32, name="ps_kl", tag="acc512")
            for ct in range(CT):
                nc.tensor.matmul(
                    kps[:N, :], lT[:, ct, :N], wlk[:, ct, sl],
                    start=(ct == 0), stop=(ct == CT - 1),
                )
            nc.vector.tensor_copy(kl_sb[:N, sl], kps[:N, :])
        klT = persist.tile([P, CT, N], f16)
        for dt in range(CT):
            pst = psT.tile([P, N], f16, name="pst_a", tag="tp")
            nc.tensor.transpose(
                pst[:, :], kl_sb[:N, dt * P:(dt + 1) * P], eye[:N, :N])
            nc.vector.tensor_tensor(
                klT[:, dt, :], pst[:, :],
                blk[:, dt:dt + 1].to_broadcast([P, N]), mybir.AluOpType.add)

        # c[n] = K_l[n] . b_vk  -> cT [n, 1] fp32
        cps = psT.tile([P, 1], fp32, name="ps_c", tag="tp")
        for dt in range(CT):
            nc.tensor.matmul(
                cps[:N, :], klT[:, dt, :N], bvk[:, dt:dt + 1],
                start=(dt == 0), stop=(dt == CT - 1),
            )
        cT = persist.tile([P, 1], fp32)
        nc.vector.tensor_copy(cT[:N, :], cps[:N, :])

        # M natural [n, c] = K_l @ (s*W_vk) via wide matmuls, then
        # PE-transpose to mT [c, n] tiles.
        m_sb = persist.tile([P, C], f16)
        for cc in range(2):
            sl = slice(cc * 512, (cc + 1) * 512)
            mps = psO.tile([P, 512], fp32, name="ps_m", tag="acc512")
            for dt in range(CT):
                nc.tensor.matmul(
                    mps[:N, :], klT[:, dt, :N], wvk[:, dt, sl],
                    start=(dt == 0), stop=(dt == CT - 1),
                )
            nc.vector.tensor_copy(m_sb[:N, sl], mps[:N, :])
        mT = persist.tile([P, CT, N], f16)
        for ct in range(CT):
            pst = psT.tile([P, N], f16, name="pst_m", tag="tp")
            nc.tensor.transpose(
                pst[:, :], m_sb[:N, ct * P:(ct + 1) * P], eye[:N, :N])
            nc.vector.tensor_copy(mT[:, ct, :], pst[:, :])

        # ---- persistent accumulators ---------------------------------
        rz_all = persist.tile([P, S // P], fp32)   # 1/Z, [s mod 128, s // 128]
        yp0 = psY.tile([P, 512], fp32, name="ps_y0", tag="ydd")
        yp1 = psY.tile([P, 512], fp32, name="ps_y1", tag="ydd")
        gps = psG.tile([P, 1], fp32, name="ps_g", tag="g")

        expat_tiles = []

        # ================= pass 1: over s-chunks ======================
        for ch in range(NCHUNKS):
            s0 = ch * SCHUNK
            if ch < 3:
                vt, vn = vt_pre[ch], vn_pre[ch]
            else:
                vt = io.tile([P, CT, SCHUNK], f16, name="vis_chunk",
                             tag="vis_chunk")
                for k in range(0, CT, 4):
                    nc.sync.dma_start(out=vt[:, k:k + 4, :],
                                      in_=visT_r[:, k:k + 4, s0:s0 + SCHUNK])
                vn = io2.tile([P, SBLK, C], f16, name="visn_chunk",
                              tag="visn_chunk")
                for b in range(0, SBLK, 2):
                    nc.scalar.dma_start(
                        out=vn[:, b:b + 2, :],
                        in_=visN_r[:, ch * SBLK + b:ch * SBLK + b + 2, :])
                absorb(vt[:, 0, :])
                absorb(vn[:, 0, :])

            # logits [n, s-chunk]
            lg = psO.tile([P, SCHUNK], fp32, name="ps_logits", tag="acc512")
            for k in range(CT):
                nc.tensor.matmul(
                    lg[:N, :], mT[:, k, :N], vt[:, k, :],
                    start=(k == 0), stop=(k == CT - 1),
                    skip_group_check=True,
                )

            # E = exp(logits + c) kept resident for pass 2
            ea = expat_pool.tile([P, SCHUNK], f16, name="expat")
            nc.scalar.activation(ea[:N, :], lg[:N, :], EXP, bias=cT[:N, 0:1])

            # per 128-row block: transpose -> [s, n], Z, 1/Z, A=E/Z,
            # then Y += A-block^T-stationary @ visN, g += A @ 1
            for b in range(SBLK):
                pst = psT.tile([P, N], f16, name="pst_a", tag="tp")
                nc.tensor.transpose(
                    pst[:, :], ea[:N, b * P:(b + 1) * P], eye[:N, :N]
                )
                zcol = work.tile([P, 1], fp32, name="zcol")
                nc.vector.reduce_sum(zcol[:], pst[:, :], axis=mybir.AxisListType.X)
                rzc = rz_all[:, ch * SBLK + b: ch * SBLK + b + 1]
                nc.vector.reciprocal(rzc, zcol[:])
                an = work.tile([P, N], f16, name="a_norm")
                nc.vector.tensor_tensor(
                    an[:, :], pst[:, :], rzc.to_broadcast([P, N]),
                    mybir.AluOpType.mult)
                first = (ch == 0 and b == 0)
                last = (ch == NCHUNKS - 1 and b == SBLK - 1)
                nc.tensor.matmul(
                    yp0[:N, :], an[:, :], vn[:, b, 0:512],
                    start=first, stop=last, skip_group_check=True)
                nc.tensor.matmul(
                    yp1[:N, :], an[:, :], vn[:, b, 512:1024],
                    start=first, stop=last, skip_group_check=True)
                nc.tensor.matmul(
                    gps[:N, :], an[:, :], ones[:, :],
                    start=first, stop=last, skip_group_check=True)

            expat_tiles.append(ea)

            if ch == 5:
                # V_l [n, c] natural, +b_lv — interleaved here so the
                # epilogue never waits on it (W_lv arrived mid-pass-1)
                vl = persist.tile([P, C], fp32)
                for cc in range(2):
                    sl = slice(cc * 512, (cc + 1) * 512)
                    vps = psO.tile([P, 512], fp32, name="ps_vl", tag="acc512")
                    for dt in range(CT):
                        nc.tensor.matmul(
                            vps[:N, :], lT[:, dt, :N], wlv[:, dt, sl],
                            start=(dt == 0), stop=(dt == CT - 1),
                            skip_group_check=True,
                        )
                    nc.vector.tensor_add(vl[:N, sl], vps[:N, :], blv[:N, sl])

        # ============ epilogue: X = Y @ W_vv.T + g (x) b_vv ===========
        # Y psum -> SBUF, transpose to [d, n]
        y_sb = persist.tile([P, C], f16)
        nc.vector.tensor_copy(y_sb[:N, 0:512], yp0[:N, :])
        nc.vector.tensor_copy(y_sb[:N, 512:1024], yp1[:N, :])
        g_sb = persist.tile([P, 1], f16)
        nc.vector.tensor_copy(g_sb[:N, :], gps[:N, :])

        yT = persist.tile([P, CT, N], f16)
        for t in range(CT):
            pst = psT.tile([P, N], f16, name="pst_a", tag="tp")
            nc.tensor.transpose(
                pst[:, :], y_sb[:N, t * P:(t + 1) * P], eye[:N, :N]
            )
            nc.vector.tensor_copy(yT[:, t, :], pst[:, :])
        pstg = psT.tile([P, N], f16, name="pst_a", tag="tp")
        nc.tensor.transpose(pstg[:1, :], g_sb[:N, 0:1], eye[:N, :N])
        gT = persist.tile([1, N], f16)
        nc.vector.tensor_copy(gT[:1, :], pstg[:1, :])

        wx = persist.tile([P, C], f16)
        for cc in range(2):
            sl = slice(cc * 512, (cc + 1) * 512)
            xps = psY.tile([P, 512], fp32, name="ps_x", tag="ydd")
            for t in range(CT):
                nc.tensor.matmul(
                    xps[:N, :], yT[:, t, :N], wvv[:, t, sl],
                    start=(t == 0), stop=False, skip_group_check=True,
                )
            nc.tensor.matmul(
                xps[:N, :], gT[:1, :N], bvv[:1, sl],
                start=False, stop=True, skip_group_check=True,
            )
            # wx = V_l + X
            nc.vector.tensor_tensor(
                wx[:N, sl], vl[:N, sl], xps[:N, :], mybir.AluOpType.add)

        # ================= pass 2: out = (E @ wx) / Z =================
        for ch in range(NCHUNKS):
            ea = expat_tiles[ch]
            for b in range(SBLK):
                rzc = rz_all[:, ch * SBLK + b: ch * SBLK + b + 1]
                r0 = ch * SCHUNK + b * P
                mid = work.tile([P, C], f16, name="mid_out", bufs=6)
                for cc in range(2):
                    sl = slice(cc * 512, (cc + 1) * 512)
                    if cc == 0:
                        ops_ = psO.tile([P, 512], fp32, name="ps_out",
                                        tag="acc512")
                    else:
                        ops_ = psY.tile([P, 512], fp32, name="ps_out2",
                                        tag="ydd")
                    nc.tensor.matmul(
                        ops_[:, :], ea[:N, b * P:(b + 1) * P], wx[:N, sl],
                        start=True, stop=True,
                    )
                    # 1/Z scale on the copyout; alternate ACT/DVE
                    if cc == 0:
                        nc.scalar.activation(mid[:, sl], ops_[:, :], COPY,
                                             scale=rzc)
                    else:
                        nc.vector.tensor_tensor(
                            mid[:, sl], ops_[:, :], rzc.to_broadcast([P, 512]),
                            mybir.AluOpType.mult)
                    # 128KB store per half; alternate queue sets for
                    # transfer parallelism
                    eng = nc.sync if cc == 0 else nc.gpsimd
                    eng.dma_start(out=out_d[r0:r0 + P, sl], in_=mid[:, sl])

    nc.compile()
    _prog_cache["nc"] = nc
    return nc


def _make_in_maps(inputs):
    vis_features = inputs["vis_features"]
    lang_features = inputs["lang_features"]
    W_vk, b_vk = inputs["W_vk"], inputs["b_vk"]
    W_lk, b_lk = inputs["W_lk"], inputs["b_lk"]
    W_vv, b_vv = inputs["W_vv"], inputs["b_vv"]
    W_lv, b_lv = inputs["W_lv"], inputs["b_lv"]
    assert vis_features.shape == (B, S, C) and lang_features.shape == (B, N, C)

    f = np.float32
    scale = f(C) ** f(-0.5)  # 2**-5, exact
    h = np.float16
    wlkTh = np.ascontiguousarray(W_lk.T.astype(f)).astype(h)
    wvkNh = np.ascontiguousarray((W_vk * scale).astype(f)).astype(h)
    wvvTh = np.ascontiguousarray(W_vv.T.astype(f)).astype(h)
    wlvTh = np.ascontiguousarray(W_lv.T.astype(f)).astype(h)
    bvk_c = np.ascontiguousarray((b_vk * scale).astype(f).reshape(CT, P).T).astype(h)
    blk_c = np.ascontiguousarray(b_lk.astype(f).reshape(CT, P).T)
    blv_b = np.ascontiguousarray(np.broadcast_to(b_lv.astype(f), (P, C)))
    bvv_r = b_vv.astype(f).reshape(1, C).astype(h)
    ones_col = np.ones((P, 1), dtype=h)
    eye = np.eye(P, dtype=h)

    shared = dict(wlkT=wlkTh, wvkN=wvkNh, wvvT=wvvTh, wlvT=wlvTh,
                  bvk_c=bvk_c, blk_c=blk_c, blv_b=blv_b, bvv_r=bvv_r,
                  ones_h=ones_col, eye=eye)
    in_maps = []
    for b in range(B):
        m = dict(shared)
        vb = vis_features[b].astype(f)
        m["visT"] = np.ascontiguousarray(vb.T).astype(h)
        m["visN"] = np.ascontiguousarray(vb).astype(h)
        m["langT"] = np.ascontiguousarray(lang_features[b].T.astype(f)).astype(h)
        in_maps.append(m)
    return in_maps


def kernel(**inputs):
    in_maps = _make_in_maps(inputs)
    nc = _build_program()
    from concourse.bass_utils import run_bass_kernel_spmd
    res = run_bass_kernel_spmd(nc, in_maps, list(range(NCORES)))
    return np.stack(
        [res.results[i]["out"].astype(np.float32) for i in range(NCORES)],
        axis=0)
